# revision 3
# baseline (speedup 1.0000x reference)
"""Trainium2 Bass kernel for nn_CausalSelfAttention_38620345926298.

Sharding: 8 cores = 4 batches x 2 head-groups (8 heads each).
Device layout: attention computed transposed, attT[s, t] (key index s on
partitions, query index t on free dim), so h, q^T, k^T, v all load/consume in
natural orientation and no on-device transposes are needed.

Per-core device program (SPMD):
  phase 1: q^T = (Wq/8)^T x^T, k^T = Wk^T x^T  (c_out on partitions, +bias via
           ACT), v = x Wv (t on partitions) with a ones column appended.
  phase 2: per head: attT = k q^T (PE), += h^T (DVE, PSUM), *= blur masks
           (DVE, sub-regions), exp (ACT, PSUM->SBUF);
           y^T(65 rows) = [v|1]^T att_exp accumulated over s-tiles -- row 64 is
           the softmax denominator. recip -> broadcast via K=1 outer-product
           matmul -> y^T *= recip.
  phase 3: out^T = Wp_slice^T y^T -> DRAM (host sums core pairs, transposes,
           adds bv@Wp + bp).

Causal mask is exact: host pre-adds -1e30 to the lower-left of diagonal
128-blocks of h^T; sub-diagonal blocks are never computed.
Softmax skips max-subtraction (logits are O(1) here; exp cannot overflow).
"""

import numpy as np

B, T, C = 4, 827, 1024
NH, HD = 16, 64
NCORES = 8
HPG = NH // 2          # heads per group (per core)
GW = HPG * HD          # group width = 512
PT = 128               # partition tile
NT = (T + PT - 1) // PT  # 7 t/s tiles
KT = C // PT           # 8 k tiles
BANK = 512             # psum bank, f32 elems
NEG = -1.0e30

F32R = False           # use float32r (full-rate) matmuls for the big GEMMs

_CACHE = {}


def _tsz(i):
    return min(PT, T - i * PT)


def _chunks(t0):
    """Bank-aligned free-dim chunks covering [t0, T)."""
    out = []
    if t0 < BANK:
        out.append((t0, BANK - t0))
        out.append((BANK, T - BANK))
    else:
        out.append((t0, T - t0))
    return out


def _build_nc():
    import concourse.tile as tile
    import concourse.mybir as mybir
    from concourse import bacc

    f32 = mybir.dt.float32
    mdt = mybir.dt.float32r if F32R else mybir.dt.float32

    def mm(ap):
        return ap

    nc = bacc.Bacc("TRN2", target_bir_lowering=False, debug=False,
                   num_devices=NCORES)

    xT = nc.dram_tensor("xT", [C, T], mdt, kind="ExternalInput").ap()
    wq = nc.dram_tensor("wq", [C, GW], mdt, kind="ExternalInput").ap()
    wk = nc.dram_tensor("wk", [C, GW], mdt, kind="ExternalInput").ap()
    wv = nc.dram_tensor("wv", [C, GW], mdt, kind="ExternalInput").ap()
    wp = nc.dram_tensor("wp", [GW, C], mdt, kind="ExternalInput").ap()
    bq = nc.dram_tensor("bq", [GW, 1], f32, kind="ExternalInput").ap()
    bk = nc.dram_tensor("bk", [GW, 1], f32, kind="ExternalInput").ap()
    hT = nc.dram_tensor("hT", [HPG, T, T], f32, kind="ExternalInput").ap()
    m01 = nc.dram_tensor("m01", [2, PT, 256], f32, kind="ExternalInput").ap()
    m02 = nc.dram_tensor("m02", [2, PT, 256], f32, kind="ExternalInput").ap()
    m12 = nc.dram_tensor("m12", [3, PT, 256], f32, kind="ExternalInput").ap()
    outT = nc.dram_tensor("outT", [C, T], f32, kind="ExternalOutput").ap()

    Exp = mybir.ActivationFunctionType.Exp

    with tile.TileContext(nc) as tc:
        with tc.tile_pool(name="persist", bufs=1) as persist:
            # ---- constants / persistent tiles ----
            ones64 = persist.tile([1, HD], f32, tag="ones64")
            nc.vector.memset(ones64, 1.0)
            msk = {}
            for mname, map_, nblk in (("m01", m01, 2), ("m02", m02, 2),
                                      ("m12", m12, 3)):
                for j in range(nblk):
                    mt = persist.tile([PT, 256], f32, name=f"{mname}_{j}",
                                      tag=f"{mname}_{j}")
                    nc.sync.dma_start(out=mt[:], in_=map_[j])
                    msk[(mname, j)] = mt

            qT = [persist.tile([PT, T], mdt, name=f"qT{m}", tag=f"qT{m}")
                  for m in range(GW // PT)]
            kTt = [persist.tile([PT, T], mdt, name=f"kT{m}", tag=f"kT{m}")
                   for m in range(GW // PT)]
            vt = [persist.tile([PT, HPG, HD + 1], mdt, name=f"v{t}",
                               tag=f"v{t}") for t in range(NT)]
            yT = [persist.tile([PT, T], mdt, name=f"yT{m}", tag=f"yT{m}")
                  for m in range(GW // PT)]

            # ================= phase 1: projections =================
            with tc.tile_pool(name="p1", bufs=1) as p1, \
                 tc.tile_pool(name="p1p", bufs=2, space="PSUM") as p1p, \
                 tc.tile_pool(name="p1vp", bufs=2, space="PSUM") as p1vp:
                xt = [p1.tile([PT, T], mdt, name=f"xt{k}", tag=f"xt{k}")
                      for k in range(KT)]
                for k in range(KT):
                    nc.sync.dma_start(out=xt[k][:], in_=xT[k * PT:(k + 1) * PT, :])
                wts = {}
                for wname, wap in (("wq", wq), ("wk", wk), ("wv", wv)):
                    wts[wname] = []
                    for k in range(KT):
                        wtile = p1.tile([PT, GW], mdt, name=f"{wname}_{k}",
                                        tag=f"{wname}_{k}")
                        nc.sync.dma_start(out=wtile[:],
                                          in_=wap[k * PT:(k + 1) * PT, :])
                        wts[wname].append(wtile)
                bqs, bks = [], []
                for m in range(GW // PT):
                    bt = p1.tile([PT, 1], f32, name=f"bq_{m}", tag=f"bq_{m}")
                    nc.sync.dma_start(out=bt[:], in_=bq[m * PT:(m + 1) * PT, :])
                    bqs.append(bt)
                    bt2 = p1.tile([PT, 1], f32, name=f"bk_{m}", tag=f"bk_{m}")
                    nc.sync.dma_start(out=bt2[:], in_=bk[m * PT:(m + 1) * PT, :])
                    bks.append(bt2)

                # q^T / k^T: out (128, T) per m-tile, contraction over C
                for wname, dest, biases in (("wq", qT, bqs), ("wk", kTt, bks)):
                    for m in range(GW // PT):
                        ps = p1p.tile([PT, T], f32, tag="proj")
                        for (c0, cn) in ((0, BANK), (BANK, T - BANK)):
                            for k in range(KT):
                                nc.tensor.matmul(
                                    ps[:, c0:c0 + cn],
                                    mm(wts[wname][k][:, m * PT:(m + 1) * PT]),
                                    mm(xt[k][:, c0:c0 + cn]),
                                    start=(k == 0), stop=(k == KT - 1))
                        nc.scalar.add(dest[m][:], ps[:], biases[m][:])

                # v: out (tsz, 512) per t-tile; append ones column
                for t in range(NT):
                    tsz = _tsz(t)
                    ps = p1vp.tile([PT, GW], f32, tag="vproj")
                    for k in range(KT):
                        nc.tensor.matmul(
                            ps[:tsz, :],
                            mm(xt[k][:, t * PT:t * PT + tsz]),
                            mm(wts["wv"][k][:]),
                            start=(k == 0), stop=(k == KT - 1))
                    nc.vector.memset(vt[t][:, :, HD:HD + 1], 1.0)
                    nc.vector.tensor_copy(
                        vt[t][:tsz, :, 0:HD],
                        ps[:tsz, :].rearrange("p (h d) -> p h d", h=HPG))

            # ================= phase 2: attention =================
            with tc.tile_pool(name="p2", bufs=1) as p2, \
                 tc.tile_pool(name="ht", bufs=4) as htp, \
                 tc.tile_pool(name="ax", bufs=3) as axp, \
                 tc.tile_pool(name="bc", bufs=2) as bcp, \
                 tc.tile_pool(name="rc", bufs=2) as rcp, \
                 tc.tile_pool(name="attp", bufs=2, space="PSUM") as attp, \
                 tc.tile_pool(name="yp", bufs=2, space="PSUM") as yp:

                def head_slices(h):
                    mt = h // 2
                    p0 = (h % 2) * HD
                    return mt, p0

                def att_stage(h, s, y_ps):
                    """att matmul + h-add + masks + exp + y accumulate for one
                    (head, s-tile)."""
                    mt, p0 = head_slices(h)
                    ssz = _tsz(s)
                    t0 = s * PT
                    L = T - t0
                    a_ps = attp.tile([PT, T], f32, tag="att")
                    for (c0, cn) in _chunks(t0):
                        nc.tensor.matmul(
                            a_ps[:ssz, c0:c0 + cn],
                            mm(kTt[mt][p0:p0 + HD, t0:t0 + ssz]),
                            mm(qT[mt][p0:p0 + HD, c0:c0 + cn]),
                            start=True, stop=True)
                    ht_t = htp.tile([PT, T], f32, tag="ht")
                    nc.sync.dma_start(out=ht_t[:ssz, :L],
                                      in_=hT[h, t0:t0 + ssz, t0:T])
                    nc.vector.tensor_add(a_ps[:ssz, t0:T], a_ps[:ssz, t0:T],
                                         ht_t[:ssz, :L])
                    # blur-mask regions (in PSUM, global t coords)
                    if s in (0, 1):
                        nc.vector.tensor_mul(a_ps[:, 285:541], a_ps[:, 285:541],
                                             msk[("m01", s)][:])
                        nc.vector.tensor_mul(a_ps[:, 571:T], a_ps[:, 571:T],
                                             msk[("m02", s)][:])
                    elif s in (2, 3):
                        nc.vector.tensor_mul(a_ps[:, 571:T], a_ps[:, 571:T],
                                             msk[("m12", s - 2)][:])
                    elif s == 4:
                        nc.vector.tensor_mul(a_ps[0:32, 571:T],
                                             a_ps[0:32, 571:T],
                                             msk[("m12", 2)][0:32, :])
                    a_sb = axp.tile([PT, T], mdt, tag="ax")
                    nc.scalar.activation(a_sb[:ssz, 0:L], a_ps[:ssz, t0:T], Exp)
                    # last s-tile contributing to bank 0 is s=3 (t0=384<512);
                    # bank 1's last is s=NT-1 — stop must close each bank.
                    for (c0, cn) in _chunks(t0):
                        last_s = (BANK // PT - 1) if c0 < BANK else (NT - 1)
                        nc.tensor.matmul(
                            y_ps[:, c0:c0 + cn],
                            mm(vt[s][:ssz, h % HPG, :]),
                            mm(a_sb[:ssz, c0 - t0:c0 - t0 + cn]),
                            start=(s == 0), stop=(s == last_s))

                def head_tail(h, y_ps):
                    mt, p0 = head_slices(h)
                    recip = rcp.tile([1, T], f32, tag="rc")
                    nc.vector.reciprocal(recip[:], y_ps[HD:HD + 1, :])
                    b_ps = attp.tile([HD, T], f32, tag="att")
                    for (c0, cn) in _chunks(0):
                        nc.tensor.matmul(b_ps[:, c0:c0 + cn], ones64[:],
                                         recip[:, c0:c0 + cn],
                                         start=True, stop=True)
                    b_sb = bcp.tile([HD, T], f32, tag="bc")
                    nc.scalar.copy(b_sb[:], b_ps[:])
                    nc.vector.tensor_mul(yT[mt][p0:p0 + HD, :], y_ps[0:HD, :],
                                         b_sb[:])

                for hp in range(HPG // 2):
                    hA, hB = 2 * hp, 2 * hp + 1
                    yA = yp.tile([HD + 1, T], f32, tag="y")
                    yB = yp.tile([HD + 1, T], f32, tag="y")
                    for s in range(NT):
                        att_stage(hA, s, yA)
                        att_stage(hB, s, yB)
                    head_tail(hA, yA)
                    head_tail(hB, yB)

            # ================= phase 3: output projection =================
            with tc.tile_pool(name="p3", bufs=1) as p3, \
                 tc.tile_pool(name="p3o", bufs=2) as p3o, \
                 tc.tile_pool(name="p3p", bufs=2, space="PSUM") as p3p:
                wpt = [p3.tile([PT, C], mdt, name=f"wp{k}", tag=f"wp{k}")
                       for k in range(GW // PT)]
                for k in range(GW // PT):
                    nc.sync.dma_start(out=wpt[k][:],
                                      in_=wp[k * PT:(k + 1) * PT, :])
                for m in range(C // PT):
                    ps = p3p.tile([PT, T], f32, tag="op")
                    for (c0, cn) in ((0, BANK), (BANK, T - BANK)):
                        for k in range(GW // PT):
                            nc.tensor.matmul(
                                ps[:, c0:c0 + cn],
                                mm(wpt[k][:, m * PT:(m + 1) * PT]),
                                mm(yT[k][:, c0:c0 + cn]),
                                start=(k == 0), stop=(k == GW // PT - 1))
                    ot = p3o.tile([PT, T], f32, tag="ot")
                    nc.scalar.copy(ot[:], ps[:])
                    nc.sync.dma_start(out=outT[m * PT:(m + 1) * PT, :],
                                      in_=ot[:])

    nc.compile()
    return nc


# ---------------- host-side preprocessing ----------------

def _gauss_A():
    hx = np.arange(7, dtype=np.float32) - 3.0
    k1 = np.exp(-0.5 * (hx / 1.5) ** 2)
    k1 = (k1 / k1.sum()).astype(np.float32)
    A = np.zeros((16, 16), np.float32)
    for i in range(16):
        for u in range(7):
            p = i - 3 + u
            if p < 0:
                p = -p
            if p > 15:
                p = 30 - p
            A[i, p] += k1[u]
    return A


def _blurred_map(f, b_perm):
    # f, b_perm: (B, 256, 256) -> reference's _blurred_map in numpy
    A = _gauss_A()
    bi = (f * b_perm).reshape(B * 256, 16, 16)
    bl = np.einsum("ij,njk,lk->nil", A, bi, A, optimize=True).astype(np.float32)
    mn, mx = bl.min(), bl.max()
    bl = np.clip((bl - mn) / (mx - mn), 0.0, 1.0)
    return bl.reshape(B, 256, 256) * f * b_perm


def _prep_inputs(x, h, f01, f02, f12, b01, b02, b12,
                 Wq, bq, Wk, bk, Wv, bv, Wp, bp):
    blur01 = _blurred_map(f01, np.transpose(b01, (0, 2, 1)))
    blur02 = _blurred_map(f02, np.transpose(b02, (0, 2, 1)))
    blur12 = _blurred_map(f12, np.transpose(b12, (0, 2, 1)))

    # causal -1e30 additions inside diagonal 128-blocks of h^T
    hTfull = np.ascontiguousarray(np.transpose(h, (0, 1, 3, 2)))
    for i in range(NT):
        n = _tsz(i)
        blk = np.tril(np.full((n, n), NEG, np.float32), -1)
        hTfull[:, :, i * PT:i * PT + n, i * PT:i * PT + n] += blk

    in_maps = []
    for c in range(NCORES):
        b, g = c // 2, c % 2
        sl = slice(g * GW, (g + 1) * GW)
        m12p = np.ones((384, 256), np.float32)
        m12p[30:286, :] = blur12[b].T
        in_maps.append({
            "xT": np.ascontiguousarray(x[b].T),
            "wq": np.ascontiguousarray(Wq[:, sl]) / 8.0,
            "wk": np.ascontiguousarray(Wk[:, sl]),
            "wv": np.ascontiguousarray(Wv[:, sl]),
            "wp": np.ascontiguousarray(Wp[sl, :]),
            "bq": (bq[sl] / 8.0).reshape(GW, 1).astype(np.float32),
            "bk": bk[sl].reshape(GW, 1).astype(np.float32),
            "hT": np.ascontiguousarray(hTfull[b, g * HPG:(g + 1) * HPG]),
            "m01": np.ascontiguousarray(blur01[b].T.reshape(2, PT, 256)),
            "m02": np.ascontiguousarray(blur02[b].T.reshape(2, PT, 256)),
            "m12": np.ascontiguousarray(m12p.reshape(3, PT, 256)),
        })
    return in_maps


def _postprocess(results, Wv_bias_row):
    out = np.empty((B, T, C), np.float32)
    for b in range(B):
        acc = results[2 * b]["outT"] + results[2 * b + 1]["outT"]
        out[b] = acc.T + Wv_bias_row
    return out


def kernel(**inputs):
    inputs = {k: np.asarray(v, dtype=np.float32) for k, v in inputs.items()}
    if "nc" not in _CACHE:
        _CACHE["nc"] = _build_nc()
    nc = _CACHE["nc"]

    in_maps = _prep_inputs(**inputs)
    from concourse import bass_utils
    res = bass_utils.run_bass_kernel_spmd(nc, in_maps,
                                          core_ids=list(range(NCORES)))
    row = inputs["bv"] @ inputs["Wp"] + inputs["bp"]
    return _postprocess(res.results, row.astype(np.float32))


# revision 5
# speedup vs baseline: 1.1220x; 1.1220x over previous
"""Trainium2 Bass kernel for nn_CausalSelfAttention_38620345926298.

Sharding: 8 cores = 4 batches x 2 head-groups (8 heads each).
Device layout: attention computed transposed, attT[s, t] (key index s on
partitions, query index t on free dim), so h, q^T, k^T, v all load/consume in
natural orientation and no on-device transposes are needed.

Per-core device program (SPMD):
  phase 1: q^T = (Wq/8)^T x^T, k^T = Wk^T x^T  (c_out on partitions, +bias via
           ACT), v = x Wv (t on partitions) with ones columns appended.
  phase 2: per head: attT = k q^T (PE), += h^T (DVE, PSUM), *= blur masks
           (DVE, sub-regions), exp (ACT, PSUM->SBUF);
           y^T(66 rows) = [v|1|1]^T att_exp accumulated over s-tiles -- row 64
           is the softmax denominator. recip -> broadcast via K=1 outer-product
           matmul (exact fp32) -> y^T *= recip.
  phase 3: out^T = Wp_slice^T y^T -> DRAM (host sums core pairs, transposes,
           adds bv@Wp + bp).

float32r matmuls (full PE rate) need even free-dim counts, so the t/s axes are
padded to TP=828 on device; the virtual row s=827 is killed via h^T row 827 =
-1e30 (exp -> 0) and column t=827 is never stored.

Causal mask is exact: host pre-adds -1e30 to the lower-left of diagonal
128-blocks of h^T; sub-diagonal blocks are never computed.
Softmax skips max-subtraction (logits are O(1) here; exp cannot overflow).
"""

import numpy as np

B, T, C = 4, 827, 1024
NH, HD = 16, 64
NCORES = 8
HPG = NH // 2          # heads per group (per core)
GW = HPG * HD          # group width = 512
PT = 128               # partition tile
TP = 828               # t/s axis padded even for fp32r matmuls
NT = (TP + PT - 1) // PT  # 7 t/s tiles
KT = C // PT           # 8 k tiles
BANK = 512             # psum bank, f32 elems
VW = HD + 2            # v row width incl. ones columns (66, even)
NEG = -1.0e30

F32R = True            # use float32r (full-rate) matmuls for the big GEMMs

_CACHE = {}


def _tsz(i):
    return min(PT, TP - i * PT)   # 128 x 6, 60


def _chunks(t0):
    """Bank-aligned free-dim chunks covering [t0, TP); all sizes even."""
    out = []
    if t0 < BANK:
        out.append((t0, BANK - t0))
        out.append((BANK, TP - BANK))
    else:
        out.append((t0, TP - t0))
    return out


def _build_nc():
    import concourse.tile as tile
    import concourse.mybir as mybir
    from concourse import bacc

    f32 = mybir.dt.float32
    mdt = mybir.dt.float32r if F32R else mybir.dt.float32

    nc = bacc.Bacc("TRN2", target_bir_lowering=False, debug=False,
                   num_devices=NCORES)

    xT = nc.dram_tensor("xT", [C, T], mdt, kind="ExternalInput").ap()
    wq = nc.dram_tensor("wq", [C, GW], mdt, kind="ExternalInput").ap()
    wk = nc.dram_tensor("wk", [C, GW], mdt, kind="ExternalInput").ap()
    wv = nc.dram_tensor("wv", [C, GW], mdt, kind="ExternalInput").ap()
    wp = nc.dram_tensor("wp", [GW, C], mdt, kind="ExternalInput").ap()
    bq = nc.dram_tensor("bq", [GW, 1], f32, kind="ExternalInput").ap()
    bk = nc.dram_tensor("bk", [GW, 1], f32, kind="ExternalInput").ap()
    hT = nc.dram_tensor("hT", [HPG, TP, T], f32, kind="ExternalInput").ap()
    m01 = nc.dram_tensor("m01", [2, PT, 256], f32, kind="ExternalInput").ap()
    m02 = nc.dram_tensor("m02", [2, PT, 256], f32, kind="ExternalInput").ap()
    m12 = nc.dram_tensor("m12", [3, PT, 256], f32, kind="ExternalInput").ap()
    # [:, 0:16] = 1.0 (v ones cols), [:, 16:17] = 0.0 (x pad col)
    cst = nc.dram_tensor("cst", [PT, 2 * HPG + 1], mdt,
                         kind="ExternalInput").ap()
    outT = nc.dram_tensor("outT", [C, T], f32, kind="ExternalOutput").ap()

    Exp = mybir.ActivationFunctionType.Exp

    with tile.TileContext(nc) as tc:
        with tc.tile_pool(name="persist", bufs=1) as persist:
            # ---- constants / persistent tiles ----
            ones64 = persist.tile([1, HD], f32, tag="ones64")
            nc.vector.memset(ones64, 1.0)
            msk = {}
            for mname, map_, nblk in (("m01", m01, 2), ("m02", m02, 2),
                                      ("m12", m12, 3)):
                for j in range(nblk):
                    mt = persist.tile([PT, 256], f32, name=f"{mname}_{j}",
                                      tag=f"{mname}_{j}")
                    nc.sync.dma_start(out=mt[:], in_=map_[j])
                    msk[(mname, j)] = mt

            qT = [persist.tile([PT, TP], mdt, name=f"qT{m}", tag=f"qT{m}")
                  for m in range(GW // PT)]
            kTt = [persist.tile([PT, TP], mdt, name=f"kT{m}", tag=f"kT{m}")
                   for m in range(GW // PT)]
            vt = [persist.tile([PT, HPG, VW], mdt, name=f"v{t}",
                               tag=f"v{t}") for t in range(NT)]
            yT = [persist.tile([PT, TP], mdt, name=f"yT{m}", tag=f"yT{m}")
                  for m in range(GW // PT)]

            # ================= phase 1: projections =================
            with tc.tile_pool(name="p1", bufs=1) as p1, \
                 tc.tile_pool(name="p1p", bufs=2, space="PSUM") as p1p, \
                 tc.tile_pool(name="p1vp", bufs=2, space="PSUM") as p1vp:
                xt = [p1.tile([PT, TP], mdt, name=f"xt{k}", tag=f"xt{k}")
                      for k in range(KT)]
                for k in range(KT):
                    nc.sync.dma_start(out=xt[k][:, 0:T],
                                      in_=xT[k * PT:(k + 1) * PT, :])
                    nc.sync.dma_start(out=xt[k][:, T:TP],
                                      in_=cst[:, 2 * HPG:2 * HPG + 1])
                wts = {}
                for wname, wap in (("wq", wq), ("wk", wk), ("wv", wv)):
                    wts[wname] = []
                    for k in range(KT):
                        wtile = p1.tile([PT, GW], mdt, name=f"{wname}_{k}",
                                        tag=f"{wname}_{k}")
                        nc.sync.dma_start(out=wtile[:],
                                          in_=wap[k * PT:(k + 1) * PT, :])
                        wts[wname].append(wtile)
                bqs, bks = [], []
                for m in range(GW // PT):
                    bt = p1.tile([PT, 1], f32, name=f"bq_{m}", tag=f"bq_{m}")
                    nc.sync.dma_start(out=bt[:], in_=bq[m * PT:(m + 1) * PT, :])
                    bqs.append(bt)
                    bt2 = p1.tile([PT, 1], f32, name=f"bk_{m}", tag=f"bk_{m}")
                    nc.sync.dma_start(out=bt2[:], in_=bk[m * PT:(m + 1) * PT, :])
                    bks.append(bt2)
                for t in range(NT):
                    nc.sync.dma_start(
                        out=vt[t][:, :, HD:VW],
                        in_=cst[:, 0:2 * HPG].rearrange("p (h c) -> p h c",
                                                        h=HPG))

                # q^T / k^T: out (128, TP) per m-tile, contraction over C
                for wname, dest, biases in (("wq", qT, bqs), ("wk", kTt, bks)):
                    for m in range(GW // PT):
                        ps = p1p.tile([PT, TP], f32, tag="proj")
                        for (c0, cn) in _chunks(0):
                            for k in range(KT):
                                nc.tensor.matmul(
                                    ps[:, c0:c0 + cn],
                                    wts[wname][k][:, m * PT:(m + 1) * PT],
                                    xt[k][:, c0:c0 + cn],
                                    start=(k == 0), stop=(k == KT - 1))
                        nc.scalar.add(dest[m][:], ps[:], biases[m][:])

                # v: out (tsz, 512) per t-tile
                for t in range(NT):
                    tsz = _tsz(t)
                    ps = p1vp.tile([PT, GW], f32, tag="vproj")
                    for k in range(KT):
                        nc.tensor.matmul(
                            ps[:tsz, :],
                            xt[k][:, t * PT:t * PT + tsz],
                            wts["wv"][k][:],
                            start=(k == 0), stop=(k == KT - 1))
                    nc.vector.tensor_copy(
                        vt[t][:tsz, :, 0:HD],
                        ps[:tsz, :].rearrange("p (h d) -> p h d", h=HPG))

            # ================= phase 2: attention =================
            with tc.tile_pool(name="ht", bufs=4) as htp, \
                 tc.tile_pool(name="ax", bufs=3) as axp, \
                 tc.tile_pool(name="bc", bufs=2) as bcp, \
                 tc.tile_pool(name="rc", bufs=2) as rcp, \
                 tc.tile_pool(name="attp", bufs=2, space="PSUM") as attp, \
                 tc.tile_pool(name="yp", bufs=2, space="PSUM") as yp:

                def att_stage(h, s, y_ps):
                    mt, p0 = h // 2, (h % 2) * HD
                    ssz = _tsz(s)
                    t0 = s * PT
                    a_ps = attp.tile([PT, TP], f32, tag="att")
                    for (c0, cn) in _chunks(t0):
                        nc.tensor.matmul(
                            a_ps[:ssz, c0:c0 + cn],
                            kTt[mt][p0:p0 + HD, t0:t0 + ssz],
                            qT[mt][p0:p0 + HD, c0:c0 + cn],
                            start=True, stop=True)
                    ht_t = htp.tile([PT, TP], f32, tag="ht")
                    nc.sync.dma_start(out=ht_t[:ssz, 0:T - t0],
                                      in_=hT[h, t0:t0 + ssz, t0:T])
                    nc.vector.tensor_add(a_ps[:ssz, t0:T], a_ps[:ssz, t0:T],
                                         ht_t[:ssz, 0:T - t0])
                    # blur-mask regions (in PSUM, global t coords)
                    if s in (0, 1):
                        nc.vector.tensor_mul(a_ps[:, 285:541], a_ps[:, 285:541],
                                             msk[("m01", s)][:])
                        nc.vector.tensor_mul(a_ps[:, 571:T], a_ps[:, 571:T],
                                             msk[("m02", s)][:])
                    elif s in (2, 3):
                        nc.vector.tensor_mul(a_ps[:, 571:T], a_ps[:, 571:T],
                                             msk[("m12", s - 2)][:])
                    elif s == 4:
                        nc.vector.tensor_mul(a_ps[0:32, 571:T],
                                             a_ps[0:32, 571:T],
                                             msk[("m12", 2)][0:32, :])
                    a_sb = axp.tile([PT, TP], mdt, tag="ax")
                    nc.scalar.activation(a_sb[:ssz, 0:TP - t0],
                                         a_ps[:ssz, t0:TP], Exp)
                    # last s-tile contributing to bank 0 is s=3 (t0=384<512)
                    for (c0, cn) in _chunks(t0):
                        last_s = (BANK // PT - 1) if c0 < BANK else (NT - 1)
                        nc.tensor.matmul(
                            y_ps[:, c0:c0 + cn],
                            vt[s][:ssz, h % HPG, :],
                            a_sb[:ssz, c0 - t0:c0 - t0 + cn],
                            start=(s == 0), stop=(s == last_s))

                def head_tail(h, y_ps):
                    mt, p0 = h // 2, (h % 2) * HD
                    recip = rcp.tile([1, TP], f32, tag="rc")
                    nc.vector.reciprocal(recip[:], y_ps[HD:HD + 1, :])
                    b_ps = attp.tile([HD, TP], f32, tag="att")
                    for (c0, cn) in _chunks(0):
                        nc.tensor.matmul(b_ps[:, c0:c0 + cn], ones64[:],
                                         recip[:, c0:c0 + cn],
                                         start=True, stop=True)
                    b_sb = bcp.tile([HD, TP], f32, tag="bc")
                    nc.scalar.copy(b_sb[:], b_ps[:])
                    nc.vector.tensor_mul(yT[mt][p0:p0 + HD, :], y_ps[0:HD, :],
                                         b_sb[:])

                for hp in range(HPG // 2):
                    hA, hB = 2 * hp, 2 * hp + 1
                    yA = yp.tile([VW, TP], f32, tag="y")
                    yB = yp.tile([VW, TP], f32, tag="y")
                    for s in range(NT):
                        att_stage(hA, s, yA)
                        att_stage(hB, s, yB)
                    head_tail(hA, yA)
                    head_tail(hB, yB)

            # ================= phase 3: output projection =================
            with tc.tile_pool(name="p3", bufs=1) as p3, \
                 tc.tile_pool(name="p3o", bufs=2) as p3o, \
                 tc.tile_pool(name="p3p", bufs=2, space="PSUM") as p3p:
                wpt = [p3.tile([PT, C], mdt, name=f"wp{k}", tag=f"wp{k}")
                       for k in range(GW // PT)]
                for k in range(GW // PT):
                    nc.sync.dma_start(out=wpt[k][:],
                                      in_=wp[k * PT:(k + 1) * PT, :])
                for m in range(C // PT):
                    ps = p3p.tile([PT, TP], f32, tag="op")
                    for (c0, cn) in _chunks(0):
                        for k in range(GW // PT):
                            nc.tensor.matmul(
                                ps[:, c0:c0 + cn],
                                wpt[k][:, m * PT:(m + 1) * PT],
                                yT[k][:, c0:c0 + cn],
                                start=(k == 0), stop=(k == GW // PT - 1))
                    ot = p3o.tile([PT, TP], f32, tag="ot")
                    nc.scalar.copy(ot[:], ps[:])
                    nc.sync.dma_start(out=outT[m * PT:(m + 1) * PT, :],
                                      in_=ot[:, 0:T])

    nc.compile()
    return nc


# ---------------- host-side preprocessing ----------------

def _gauss_A():
    hx = np.arange(7, dtype=np.float32) - 3.0
    k1 = np.exp(-0.5 * (hx / 1.5) ** 2)
    k1 = (k1 / k1.sum()).astype(np.float32)
    A = np.zeros((16, 16), np.float32)
    for i in range(16):
        for u in range(7):
            p = i - 3 + u
            if p < 0:
                p = -p
            if p > 15:
                p = 30 - p
            A[i, p] += k1[u]
    return A


def _blurred_map(f, b_perm):
    # f, b_perm: (B, 256, 256) -> reference's _blurred_map in numpy
    A = _gauss_A()
    bi = (f * b_perm).reshape(B * 256, 16, 16)
    bl = np.einsum("ij,njk,lk->nil", A, bi, A, optimize=True).astype(np.float32)
    mn, mx = bl.min(), bl.max()
    bl = np.clip((bl - mn) / (mx - mn), 0.0, 1.0)
    return bl.reshape(B, 256, 256) * f * b_perm


def _prep_inputs(x, h, f01, f02, f12, b01, b02, b12,
                 Wq, bq, Wk, bk, Wv, bv, Wp, bp):
    blur01 = _blurred_map(f01, np.transpose(b01, (0, 2, 1)))
    blur02 = _blurred_map(f02, np.transpose(b02, (0, 2, 1)))
    blur12 = _blurred_map(f12, np.transpose(b12, (0, 2, 1)))

    # h^T padded to TP rows; row T (virtual s) = -1e30 so exp -> 0.
    # causal -1e30 additions inside diagonal 128-blocks of h^T.
    hTfull = np.full((B, NH, TP, T), NEG, np.float32)
    hTfull[:, :, :T, :] = np.transpose(h, (0, 1, 3, 2))
    for i in range(NT):
        n = min(PT, T - i * PT)
        blk = np.tril(np.full((n, n), NEG, np.float32), -1)
        hTfull[:, :, i * PT:i * PT + n, i * PT:i * PT + n] += blk

    cstv = np.zeros((PT, 2 * HPG + 1), np.float32)
    cstv[:, 0:2 * HPG] = 1.0

    in_maps = []
    for c in range(NCORES):
        b, g = c // 2, c % 2
        sl = slice(g * GW, (g + 1) * GW)
        m12p = np.ones((384, 256), np.float32)
        m12p[30:286, :] = blur12[b].T
        in_maps.append({
            "xT": np.ascontiguousarray(x[b].T),
            "wq": np.ascontiguousarray(Wq[:, sl]) / 8.0,
            "wk": np.ascontiguousarray(Wk[:, sl]),
            "wv": np.ascontiguousarray(Wv[:, sl]),
            "wp": np.ascontiguousarray(Wp[sl, :]),
            "bq": (bq[sl] / 8.0).reshape(GW, 1).astype(np.float32),
            "bk": bk[sl].reshape(GW, 1).astype(np.float32),
            "hT": np.ascontiguousarray(hTfull[b, g * HPG:(g + 1) * HPG]),
            "m01": np.ascontiguousarray(blur01[b].T.reshape(2, PT, 256)),
            "m02": np.ascontiguousarray(blur02[b].T.reshape(2, PT, 256)),
            "m12": np.ascontiguousarray(m12p.reshape(3, PT, 256)),
            "cst": cstv,
        })
    return in_maps


def _postprocess(results, Wv_bias_row):
    out = np.empty((B, T, C), np.float32)
    for b in range(B):
        acc = results[2 * b]["outT"] + results[2 * b + 1]["outT"]
        out[b] = acc.T + Wv_bias_row
    return out


def kernel(**inputs):
    inputs = {k: np.asarray(v, dtype=np.float32) for k, v in inputs.items()}
    if "nc" not in _CACHE:
        _CACHE["nc"] = _build_nc()
    nc = _CACHE["nc"]

    in_maps = _prep_inputs(**inputs)
    from concourse import bass_utils
    res = bass_utils.run_bass_kernel_spmd(nc, in_maps,
                                          core_ids=list(range(NCORES)))
    row = inputs["bv"] @ inputs["Wp"] + inputs["bp"]
    return _postprocess(res.results, row.astype(np.float32))


# revision 17
# speedup vs baseline: 1.1877x; 1.0586x over previous
"""Trainium2 Bass kernel for nn_CausalSelfAttention_38620345926298.

Sharding: 8 cores = 4 batches x 2 head-groups (8 heads each).
Device layout: attention computed transposed, attT[s, t] (key index s on
partitions, query index t on free dim), so h, q^T, k^T, v all load/consume in
natural orientation and no on-device transposes are needed.

Per-core device program (SPMD):
  phase 1: q^T = (Wq/8)^T x^T, k^T = Wk^T x^T  (c_out on partitions, +bias via
           ACT), v = x Wv (t on partitions) with ones columns appended.
  phase 2: per head: attT = k q^T (PE), += h^T (DVE, PSUM), *= blur masks
           (DVE, sub-regions), exp (ACT, PSUM->SBUF);
           y^T(66 rows) = [v|1|1]^T att_exp accumulated over s-tiles -- row 64
           is the softmax denominator. recip -> broadcast via K=1 outer-product
           matmul (exact fp32) -> y^T *= recip.
  phase 3: out^T = Wp_slice^T y^T -> DRAM (host sums core pairs, transposes,
           adds bv@Wp + bp).

float32r matmuls (full PE rate) need even free-dim counts, so the t/s axes are
padded to TP=828 on device; the virtual row s=827 is killed via h^T row 827 =
-1e30 (exp -> 0) and column t=827 is never stored.

Causal mask is exact: host pre-adds -1e30 to the lower-left of diagonal
128-blocks of h^T; sub-diagonal blocks are never computed.
Softmax skips max-subtraction (logits are O(1) here; exp cannot overflow).
"""

import numpy as np

B, T, C = 4, 827, 1024
NH, HD = 16, 64
NCORES = 8
HPG = NH // 2          # heads per group (per core)
GW = HPG * HD          # group width = 512
PT = 128               # partition tile
TP = 828               # t/s axis padded even for fp32r matmuls
NT = (TP + PT - 1) // PT  # 7 t/s tiles
KT = C // PT           # 8 k tiles
BANK = 512             # psum bank, f32 elems
VW = HD + 2            # v row width incl. ones columns (66, even)
NEG = -1.0e30

F32R = True            # use float32r (full-rate) matmuls for the big GEMMs

_CACHE = {}


def _tsz(i):
    return min(PT, TP - i * PT)   # 128 x 6, 60


def _chunks(t0):
    """Bank-aligned free-dim chunks covering [t0, TP); all sizes even."""
    out = []
    if t0 < BANK:
        out.append((t0, BANK - t0))
        out.append((BANK, TP - BANK))
    else:
        out.append((t0, TP - t0))
    return out


WIDE = True
YNARROW = False
QNARROW = False
# Widening s>=4 (base 512 < t0) triggers a hardware fault in the fp32r qk
# matmul (bisected: lhsT offset 2560/3072 + rhs offset 2048 + dst 0 on the
# K=64 att matmul dies; same shapes at s=4 work). s=3 widening is verified.
WIDE_SET = frozenset([3])


def _base(t0):
    """Widened chunk start (>=256 sizes keep fp32r at full rate); columns in
    [base, t0) are sub-diagonal and get killed by h^T = -1e30 -> exp 0."""
    if not WIDE or (t0 // PT) not in WIDE_SET:
        return t0
    return min(t0, BANK - 256) if t0 < BANK else BANK


def _chunks_w(t0):
    b = _base(t0)
    if b < BANK:
        return [(b, BANK - b), (BANK, TP - BANK)]
    return [(b, TP - b)]


def _build_nc():
    import concourse.tile as tile
    import concourse.mybir as mybir
    from concourse import bacc

    f32 = mybir.dt.float32
    mdt = mybir.dt.float32r if F32R else mybir.dt.float32

    nc = bacc.Bacc("TRN2", target_bir_lowering=False, debug=False,
                   num_devices=NCORES)

    xT = nc.dram_tensor("xT", [C, T], mdt, kind="ExternalInput").ap()
    wq = nc.dram_tensor("wq", [C, GW], mdt, kind="ExternalInput").ap()
    wk = nc.dram_tensor("wk", [C, GW], mdt, kind="ExternalInput").ap()
    wv = nc.dram_tensor("wv", [C, GW], mdt, kind="ExternalInput").ap()
    wp = nc.dram_tensor("wp", [GW, C], mdt, kind="ExternalInput").ap()
    bq = nc.dram_tensor("bq", [GW, 1], f32, kind="ExternalInput").ap()
    bk = nc.dram_tensor("bk", [GW, 1], f32, kind="ExternalInput").ap()
    hT = nc.dram_tensor("hT", [HPG, TP, TP], mdt, kind="ExternalInput").ap()
    m01 = nc.dram_tensor("m01", [2, PT, 256], f32, kind="ExternalInput").ap()
    m02 = nc.dram_tensor("m02", [2, PT, 256], f32, kind="ExternalInput").ap()
    m12 = nc.dram_tensor("m12", [3, PT, 256], f32, kind="ExternalInput").ap()
    # [:, 0:HD] = 1.0 (v ones cols, ones64 row), [:, HD] = 0.0 (x pad col)
    cst = nc.dram_tensor("cst", [PT, HD + 1], mdt,
                         kind="ExternalInput").ap()
    ident = nc.dram_tensor("ident", [PT, PT], mdt, kind="ExternalInput").ap()
    outT = nc.dram_tensor("outT", [C, T], f32, kind="ExternalOutput").ap()

    Exp = mybir.ActivationFunctionType.Exp

    with tile.TileContext(nc) as tc:
        with tc.tile_pool(name="persist", bufs=1) as persist:
            # ---- constants / persistent tiles ----
            ones64 = persist.tile([1, HD], mdt, tag="ones64")
            id_sb = persist.tile([PT, PT], mdt, tag="id_sb")
            wpt = [persist.tile([PT, C], mdt, name=f"wp{k}", tag=f"wp{k}")
                   for k in range(GW // PT)]
            msk = {}
            for mname, map_, nblk in (("m01", m01, 2), ("m02", m02, 2),
                                      ("m12", m12, 3)):
                for j in range(nblk):
                    mt = persist.tile([PT, 256], f32, name=f"{mname}_{j}",
                                      tag=f"{mname}_{j}")
                    msk[(mname, j)] = mt

            def persist_dmas():
                # emitted after the phase-1 input loads so they don't delay
                # the first projection matmuls
                nc.sync.dma_start(out=ones64[:], in_=cst[0:1, 0:HD])
                nc.sync.dma_start(out=id_sb[:], in_=ident[:])
                for mname, map_, nblk in (("m01", m01, 2), ("m02", m02, 2),
                                          ("m12", m12, 3)):
                    for j in range(nblk):
                        nc.sync.dma_start(out=msk[(mname, j)][:], in_=map_[j])
                for k in range(GW // PT):
                    nc.sync.dma_start(out=wpt[k][:],
                                      in_=wp[k * PT:(k + 1) * PT, :])

            qT = [persist.tile([PT, TP], mdt, name=f"qT{m}", tag=f"qT{m}")
                  for m in range(GW // PT)]
            kTt = [persist.tile([PT, TP], mdt, name=f"kT{m}", tag=f"kT{m}")
                   for m in range(GW // PT)]
            vt = [persist.tile([PT, HPG, VW], mdt, name=f"v{t}",
                               tag=f"v{t}") for t in range(NT)]
            yT = [persist.tile([PT, TP], mdt, name=f"yT{m}", tag=f"yT{m}")
                  for m in range(GW // PT)]

            # ================= phase 1: projections =================
            with tc.tile_pool(name="p1", bufs=1) as p1, \
                 tc.tile_pool(name="p1p", bufs=2, space="PSUM") as p1p, \
                 tc.tile_pool(name="p1vp", bufs=2, space="PSUM") as p1vp:
                xt = [p1.tile([PT, TP], mdt, name=f"xt{k}", tag=f"xt{k}")
                      for k in range(KT)]
                wts = {w: [p1.tile([PT, GW], mdt, name=f"{w}_{k}",
                                   tag=f"{w}_{k}") for k in range(KT)]
                       for w in ("wq", "wk", "wv")}
                for k in range(KT):
                    nc.sync.dma_start(out=xt[k][:, 0:T],
                                      in_=xT[k * PT:(k + 1) * PT, :])
                    nc.sync.dma_start(out=xt[k][:, T:TP],
                                      in_=cst[:, HD:HD + 1])
                    for wname, wap in (("wq", wq), ("wk", wk), ("wv", wv)):
                        nc.sync.dma_start(out=wts[wname][k][:],
                                          in_=wap[k * PT:(k + 1) * PT, :])
                bqs, bks = [], []
                for m in range(GW // PT):
                    bt = p1.tile([PT, 1], f32, name=f"bq_{m}", tag=f"bq_{m}")
                    nc.sync.dma_start(out=bt[:], in_=bq[m * PT:(m + 1) * PT, :])
                    bqs.append(bt)
                    bt2 = p1.tile([PT, 1], f32, name=f"bk_{m}", tag=f"bk_{m}")
                    nc.sync.dma_start(out=bt2[:], in_=bk[m * PT:(m + 1) * PT, :])
                    bks.append(bt2)
                for t in range(NT):
                    nc.sync.dma_start(
                        out=vt[t][:, :, HD:VW],
                        in_=cst[:, 0:2 * HPG].rearrange("p (h c) -> p h c",
                                                        h=HPG))
                persist_dmas()

                # q^T / k^T: out (128, TP) per m-tile, contraction over C
                for wname, dest, biases in (("wq", qT, bqs), ("wk", kTt, bks)):
                    for m in range(GW // PT):
                        ps = p1p.tile([PT, TP], f32, tag="proj")
                        for (c0, cn) in _chunks(0):
                            for k in range(KT):
                                nc.tensor.matmul(
                                    ps[:, c0:c0 + cn],
                                    wts[wname][k][:, m * PT:(m + 1) * PT],
                                    xt[k][:, c0:c0 + cn],
                                    start=(k == 0), stop=(k == KT - 1))
                        nc.scalar.add(dest[m][:], ps[:], biases[m][:])

                # v: out (tsz, 512) per t-tile
                for t in range(NT):
                    tsz = _tsz(t)
                    ps = p1vp.tile([PT, GW], f32, tag="vproj")
                    for k in range(KT):
                        nc.tensor.matmul(
                            ps[:tsz, :],
                            xt[k][:, t * PT:t * PT + tsz],
                            wts["wv"][k][:],
                            start=(k == 0), stop=(k == KT - 1))
                    nc.vector.tensor_copy(
                        vt[t][:tsz, :, 0:HD],
                        ps[:tsz, :].rearrange("p (h d) -> p h d", h=HPG))

            # ================= phase 2: attention =================
            with tc.tile_pool(name="ht", bufs=6) as htp, \
                 tc.tile_pool(name="ax", bufs=3) as axp, \
                 tc.tile_pool(name="bc", bufs=2) as bcp, \
                 tc.tile_pool(name="rc", bufs=2) as rcp, \
                 tc.tile_pool(name="attp", bufs=4, space="PSUM") as attp, \
                 tc.tile_pool(name="yp", bufs=2, space="PSUM") as yp:

                def _mask_regions(s, c0, cn):
                    # (global_lo, global_hi, mask_tile, mask_col0, row_hi)
                    regs = []
                    if s in (0, 1):
                        regs.append((285, 541, msk[("m01", s)], 285, PT))
                        regs.append((571, T, msk[("m02", s)], 571, PT))
                    elif s in (2, 3):
                        regs.append((571, T, msk[("m12", s - 2)], 571, PT))
                    elif s == 4:
                        regs.append((571, T, msk[("m12", 2)], 571, 32))
                    out = []
                    for (lo, hi, mtile, m0, rhi) in regs:
                        a, b = max(lo, c0), min(hi, c0 + cn)
                        if a < b:
                            out.append((a, b, mtile, m0, rhi))
                    return out

                def att_stage(h, s, y_ps):
                    """One (head, s-tile), pipelined per single-bank chunk."""
                    mt, p0 = h // 2, (h % 2) * HD
                    ssz = _tsz(s)
                    t0 = s * PT
                    base = _base(t0)
                    ht_t = htp.tile([PT, TP], mdt, tag="ht")
                    nc.sync.dma_start(out=ht_t[:ssz, 0:TP - base],
                                      in_=hT[h, t0:t0 + ssz, base:TP])
                    a_sb = axp.tile([PT, TP], mdt, tag="ax")
                    for (c0, cn) in _chunks_w(t0):
                        a_ps = attp.tile([PT, BANK], f32, tag="att")
                        qc0 = max(c0, t0) if QNARROW else c0
                        nc.tensor.matmul(
                            a_ps[:ssz, qc0 - c0:cn],
                            kTt[mt][p0:p0 + HD, t0:t0 + ssz],
                            qT[mt][p0:p0 + HD, qc0:c0 + cn],
                            start=True, stop=False)
                        nc.tensor.matmul(
                            a_ps[:ssz, 0:cn],
                            id_sb[:ssz, :ssz],
                            ht_t[:ssz, c0 - base:c0 - base + cn],
                            start=False, stop=True)
                        for (a, b, mtile, m0, rhi) in _mask_regions(s, c0, cn):
                            nc.vector.tensor_mul(
                                a_ps[0:rhi, a - c0:b - c0],
                                a_ps[0:rhi, a - c0:b - c0],
                                mtile[0:rhi, a - m0:b - m0])
                        nc.scalar.activation(a_sb[:ssz, c0 - base:c0 - base + cn],
                                             a_ps[:ssz, 0:cn], Exp)
                        # bank 0 of y_ps last gets fed at s=3 (t0=384<512)
                        last_s = (BANK // PT - 1) if c0 < BANK else (NT - 1)
                        yc0 = max(c0, t0) if YNARROW else c0
                        ycn = cn - (yc0 - c0)
                        nc.tensor.matmul(
                            y_ps[:, yc0:yc0 + ycn],
                            vt[s][:ssz, h % HPG, :],
                            a_sb[:ssz, yc0 - base:yc0 - base + ycn],
                            start=(s == 0), stop=(s == last_s))

                def head_tail(h, y_ps):
                    mt, p0 = h // 2, (h % 2) * HD
                    recip = rcp.tile([1, TP], mdt, tag="rc")
                    with nc.allow_low_precision(reason="fp32r recip feeds "
                                                "full-rate fp32r bcast mm"):
                        nc.vector.reciprocal(recip[:], y_ps[HD:HD + 1, :])
                    b_sb = bcp.tile([HD, TP], f32, tag="bc")
                    for (c0, cn) in _chunks(0):
                        b_ps = attp.tile([HD, BANK], f32, tag="att")
                        nc.tensor.matmul(b_ps[:, 0:cn], ones64[:],
                                         recip[:, c0:c0 + cn],
                                         start=True, stop=True)
                        nc.scalar.copy(b_sb[:, c0:c0 + cn], b_ps[:, 0:cn])
                    nc.vector.tensor_mul(yT[mt][p0:p0 + HD, :], y_ps[0:HD, :],
                                         b_sb[:])

                for hp in range(HPG // 2):
                    hA, hB = 2 * hp, 2 * hp + 1
                    yA = yp.tile([VW, TP], f32, tag="y")
                    yB = yp.tile([VW, TP], f32, tag="y")
                    for s in range(NT):
                        att_stage(hA, s, yA)
                        att_stage(hB, s, yB)
                    head_tail(hA, yA)
                    head_tail(hB, yB)

            # ================= phase 3: output projection =================
            with tc.tile_pool(name="p3o", bufs=2) as p3o, \
                 tc.tile_pool(name="p3p", bufs=2, space="PSUM") as p3p:
                for m in range(C // PT):
                    ps = p3p.tile([PT, TP], f32, tag="op")
                    for (c0, cn) in _chunks(0):
                        for k in range(GW // PT):
                            nc.tensor.matmul(
                                ps[:, c0:c0 + cn],
                                wpt[k][:, m * PT:(m + 1) * PT],
                                yT[k][:, c0:c0 + cn],
                                start=(k == 0), stop=(k == GW // PT - 1))
                    ot = p3o.tile([PT, TP], f32, tag="ot")
                    nc.scalar.copy(ot[:], ps[:])
                    nc.sync.dma_start(out=outT[m * PT:(m + 1) * PT, :],
                                      in_=ot[:, 0:T])

    nc.compile()
    return nc


# ---------------- host-side preprocessing ----------------

def _gauss_A():
    hx = np.arange(7, dtype=np.float32) - 3.0
    k1 = np.exp(-0.5 * (hx / 1.5) ** 2)
    k1 = (k1 / k1.sum()).astype(np.float32)
    A = np.zeros((16, 16), np.float32)
    for i in range(16):
        for u in range(7):
            p = i - 3 + u
            if p < 0:
                p = -p
            if p > 15:
                p = 30 - p
            A[i, p] += k1[u]
    return A


def _blurred_map(f, b_perm):
    # f, b_perm: (B, 256, 256) -> reference's _blurred_map in numpy
    A = _gauss_A()
    bi = (f * b_perm).reshape(B * 256, 16, 16)
    bl = np.einsum("ij,njk,lk->nil", A, bi, A, optimize=True).astype(np.float32)
    mn, mx = bl.min(), bl.max()
    bl = np.clip((bl - mn) / (mx - mn), 0.0, 1.0)
    return bl.reshape(B, 256, 256) * f * b_perm


def _prep_inputs(x, h, f01, f02, f12, b01, b02, b12,
                 Wq, bq, Wk, bk, Wv, bv, Wp, bp):
    blur01 = _blurred_map(f01, np.transpose(b01, (0, 2, 1)))
    blur02 = _blurred_map(f02, np.transpose(b02, (0, 2, 1)))
    blur12 = _blurred_map(f12, np.transpose(b12, (0, 2, 1)))

    # h^T padded to TP rows/cols; the whole sub-diagonal (t < s) plus the
    # padding row/column are -1e30 so exp kills everything non-causal,
    # including sub-diagonal columns the widened fp32r chunks compute.
    hTfull = np.full((B, NH, TP, TP), NEG, np.float32)
    hTfull[:, :, :T, :T] = np.transpose(h, (0, 1, 3, 2))
    tri = np.tril(np.ones((TP, TP), dtype=bool), -1)  # t < s
    hTfull[:, :, tri] = NEG
    # padding column t=827 stays finite (exp=1) so its softmax sum is nonzero
    # and the never-stored column produces no inf/NaN downstream
    hTfull[:, :, :, T] = 0.0

    cstv = np.zeros((PT, HD + 1), np.float32)
    cstv[:, 0:HD] = 1.0
    identv = np.eye(PT, dtype=np.float32)

    in_maps = []
    for c in range(NCORES):
        b, g = c // 2, c % 2
        sl = slice(g * GW, (g + 1) * GW)
        m12p = np.ones((384, 256), np.float32)
        m12p[30:286, :] = blur12[b].T
        in_maps.append({
            "xT": np.ascontiguousarray(x[b].T),
            "wq": np.ascontiguousarray(Wq[:, sl]) / 8.0,
            "wk": np.ascontiguousarray(Wk[:, sl]),
            "wv": np.ascontiguousarray(Wv[:, sl]),
            "wp": np.ascontiguousarray(Wp[sl, :]),
            "bq": (bq[sl] / 8.0).reshape(GW, 1).astype(np.float32),
            "bk": bk[sl].reshape(GW, 1).astype(np.float32),
            "hT": np.ascontiguousarray(hTfull[b, g * HPG:(g + 1) * HPG]),
            "m01": np.ascontiguousarray(blur01[b].T.reshape(2, PT, 256)),
            "m02": np.ascontiguousarray(blur02[b].T.reshape(2, PT, 256)),
            "m12": np.ascontiguousarray(m12p.reshape(3, PT, 256)),
            "cst": cstv,
            "ident": identv,
        })
    return in_maps


def _postprocess(results, Wv_bias_row):
    out = np.empty((B, T, C), np.float32)
    for b in range(B):
        acc = results[2 * b]["outT"] + results[2 * b + 1]["outT"]
        out[b] = acc.T + Wv_bias_row
    return out


def kernel(**inputs):
    inputs = {k: np.asarray(v, dtype=np.float32) for k, v in inputs.items()}
    if "nc" not in _CACHE:
        _CACHE["nc"] = _build_nc()
    nc = _CACHE["nc"]

    in_maps = _prep_inputs(**inputs)
    from concourse import bass_utils
    res = bass_utils.run_bass_kernel_spmd(nc, in_maps,
                                          core_ids=list(range(NCORES)))
    row = inputs["bv"] @ inputs["Wp"] + inputs["bp"]
    return _postprocess(res.results, row.astype(np.float32))


# revision 24
# speedup vs baseline: 2.9308x; 2.4676x over previous
"""Trainium2 Bass kernel for nn_CausalSelfAttention_38620345926298.

Sharding: 8 cores = 4 batches x 2 head-groups (8 heads each).
Device layout: attention computed transposed, attT[s, t] (key index s on
partitions, query index t on free dim), so h, q^T, k^T, v all load/consume in
natural orientation and no on-device transposes are needed.

Per-core device program (SPMD):
  phase 1: q^T = (Wq/8)^T x^T, k^T = Wk^T x^T  (c_out on partitions, +bias via
           ACT), v = x Wv (t on partitions) with ones columns appended.
  phase 2: per head: attT = k q^T (PE), += h^T (DVE, PSUM), *= blur masks
           (DVE, sub-regions), exp (ACT, PSUM->SBUF);
           y^T(66 rows) = [v|1|1]^T att_exp accumulated over s-tiles -- row 64
           is the softmax denominator. recip -> broadcast via K=1 outer-product
           matmul (exact fp32) -> y^T *= recip.
  phase 3: out^T = Wp_slice^T y^T -> DRAM (host sums core pairs, transposes,
           adds bv@Wp + bp).

float32r matmuls (full PE rate) need even free-dim counts, so the t/s axes are
padded to TP=828 on device; the virtual row s=827 is killed via h^T row 827 =
-1e30 (exp -> 0) and column t=827 is never stored.

Causal mask is exact: host pre-adds -1e30 to the lower-left of diagonal
128-blocks of h^T; sub-diagonal blocks are never computed.
Softmax skips max-subtraction (logits are O(1) here; exp cannot overflow).
"""

import numpy as np

B, T, C = 4, 827, 1024
NH, HD = 16, 64
NCORES = 8
HPG = NH // 2          # heads per group (per core)
GW = HPG * HD          # group width = 512
PT = 128               # partition tile
TP = 828               # t/s axis padded even for fp32r matmuls
NT = (TP + PT - 1) // PT  # 7 t/s tiles
KT = C // PT           # 8 k tiles
BANK = 512             # psum bank, f32 elems
VW = HD + 2            # v row width incl. ones columns (66, even)
NEG = -1.0e30

F32R = True            # use float32r (full-rate) matmuls for the big GEMMs

_CACHE = {}


def _tsz(i):
    return min(PT, TP - i * PT)   # 128 x 6, 60


def _chunks(t0):
    """Bank-aligned free-dim chunks covering [t0, TP); all sizes even."""
    out = []
    if t0 < BANK:
        out.append((t0, BANK - t0))
        out.append((BANK, TP - BANK))
    else:
        out.append((t0, TP - t0))
    return out


WIDE = True
YNARROW = False
QNARROW = False
H_ON_DVE = True
H_BF16 = True
# Widening s>=4 (base 512 < t0) triggers a hardware fault in the fp32r qk
# matmul (bisected: lhsT offset 2560/3072 + rhs offset 2048 + dst 0 on the
# K=64 att matmul dies; same shapes at s=4 work). s=3 widening is verified.
WIDE_SET = frozenset([3])


def _base(t0):
    """Widened chunk start (>=256 sizes keep fp32r at full rate); columns in
    [base, t0) are sub-diagonal and get killed by h^T = -1e30 -> exp 0."""
    if not WIDE or (t0 // PT) not in WIDE_SET:
        return t0
    return min(t0, BANK - 256) if t0 < BANK else BANK


def _chunks_w(t0):
    b = _base(t0)
    if b < BANK:
        return [(b, BANK - b), (BANK, TP - BANK)]
    return [(b, TP - b)]


def _build_nc(loop_k=1):
    import concourse.tile as tile
    import concourse.mybir as mybir
    from concourse import bacc

    f32 = mybir.dt.float32
    mdt = mybir.dt.float32r if F32R else mybir.dt.float32

    nc = bacc.Bacc("TRN2", target_bir_lowering=False, debug=False,
                   num_devices=NCORES)

    xT = nc.dram_tensor("xT", [C, T], mdt, kind="ExternalInput").ap()
    wq = nc.dram_tensor("wq", [C, GW], mdt, kind="ExternalInput").ap()
    wk = nc.dram_tensor("wk", [C, GW], mdt, kind="ExternalInput").ap()
    wv = nc.dram_tensor("wv", [C, GW], mdt, kind="ExternalInput").ap()
    wp = nc.dram_tensor("wp", [GW, C], mdt, kind="ExternalInput").ap()
    bq = nc.dram_tensor("bq", [GW, 1], f32, kind="ExternalInput").ap()
    bk = nc.dram_tensor("bk", [GW, 1], f32, kind="ExternalInput").ap()
    hdt = mybir.dt.bfloat16 if H_BF16 else mdt
    hT = nc.dram_tensor("hT", [HPG, TP, TP], hdt, kind="ExternalInput").ap()
    m01 = nc.dram_tensor("m01", [2, PT, 256], f32, kind="ExternalInput").ap()
    m02 = nc.dram_tensor("m02", [2, PT, 256], f32, kind="ExternalInput").ap()
    m12 = nc.dram_tensor("m12", [3, PT, 256], f32, kind="ExternalInput").ap()
    # [:, 0:HD] = 1.0 (v ones cols, ones64 row), [:, HD] = 0.0 (x pad col)
    cst = nc.dram_tensor("cst", [PT, HD + 1], mdt,
                         kind="ExternalInput").ap()
    ident = nc.dram_tensor("ident", [PT, PT], mdt, kind="ExternalInput").ap()
    outT = nc.dram_tensor("outT", [C, T], f32, kind="ExternalOutput").ap()

    Exp = mybir.ActivationFunctionType.Exp

    def _emit(tc):
        with tc.tile_pool(name="persist", bufs=1) as persist:
            # ---- constants / persistent tiles ----
            ones64 = persist.tile([1, HD], mdt, tag="ones64")
            id_sb = persist.tile([PT, PT], mdt, tag="id_sb")
            wpt = [persist.tile([PT, C], mdt, name=f"wp{k}", tag=f"wp{k}")
                   for k in range(GW // PT)]
            msk = {}
            for mname, map_, nblk in (("m01", m01, 2), ("m02", m02, 2),
                                      ("m12", m12, 3)):
                for j in range(nblk):
                    mt = persist.tile([PT, 256], f32, name=f"{mname}_{j}",
                                      tag=f"{mname}_{j}")
                    msk[(mname, j)] = mt

            def persist_dmas():
                # emitted after the phase-1 input loads so they don't delay
                # the first projection matmuls
                nc.sync.dma_start(out=ones64[:], in_=cst[0:1, 0:HD])
                nc.sync.dma_start(out=id_sb[:], in_=ident[:])
                for mname, map_, nblk in (("m01", m01, 2), ("m02", m02, 2),
                                          ("m12", m12, 3)):
                    for j in range(nblk):
                        nc.sync.dma_start(out=msk[(mname, j)][:], in_=map_[j])
                for k in range(GW // PT):
                    nc.sync.dma_start(out=wpt[k][:],
                                      in_=wp[k * PT:(k + 1) * PT, :])

            qT = [persist.tile([PT, TP], mdt, name=f"qT{m}", tag=f"qT{m}")
                  for m in range(GW // PT)]
            kTt = [persist.tile([PT, TP], mdt, name=f"kT{m}", tag=f"kT{m}")
                   for m in range(GW // PT)]
            vt = [persist.tile([PT, HPG, VW], mdt, name=f"v{t}",
                               tag=f"v{t}") for t in range(NT)]
            yT = [persist.tile([PT, TP], mdt, name=f"yT{m}", tag=f"yT{m}")
                  for m in range(GW // PT)]

            # ================= phase 1: projections =================
            with tc.tile_pool(name="p1", bufs=1) as p1, \
                 tc.tile_pool(name="p1p", bufs=3, space="PSUM") as p1p, \
                 tc.tile_pool(name="p1vp", bufs=2, space="PSUM") as p1vp:
                xt = [p1.tile([PT, TP], mdt, name=f"xt{k}", tag=f"xt{k}")
                      for k in range(KT)]
                wts = {w: [p1.tile([PT, GW], mdt, name=f"{w}_{k}",
                                   tag=f"{w}_{k}") for k in range(KT)]
                       for w in ("wq", "wk", "wv")}
                for k in range(KT):
                    nc.sync.dma_start(out=xt[k][:, 0:T],
                                      in_=xT[k * PT:(k + 1) * PT, :])
                    nc.sync.dma_start(out=xt[k][:, T:TP],
                                      in_=cst[:, HD:HD + 1])
                    for wname, wap in (("wq", wq), ("wk", wk), ("wv", wv)):
                        nc.sync.dma_start(out=wts[wname][k][:],
                                          in_=wap[k * PT:(k + 1) * PT, :])
                bqs, bks = [], []
                for m in range(GW // PT):
                    bt = p1.tile([PT, 1], f32, name=f"bq_{m}", tag=f"bq_{m}")
                    nc.sync.dma_start(out=bt[:], in_=bq[m * PT:(m + 1) * PT, :])
                    bqs.append(bt)
                    bt2 = p1.tile([PT, 1], f32, name=f"bk_{m}", tag=f"bk_{m}")
                    nc.sync.dma_start(out=bt2[:], in_=bk[m * PT:(m + 1) * PT, :])
                    bks.append(bt2)
                for t in range(NT):
                    nc.sync.dma_start(
                        out=vt[t][:, :, HD:VW],
                        in_=cst[:, 0:2 * HPG].rearrange("p (h c) -> p h c",
                                                        h=HPG))
                persist_dmas()

                # q^T / k^T: out (128, TP) per m-tile, contraction over C
                for wname, dest, biases in (("wq", qT, bqs), ("wk", kTt, bks)):
                    for m in range(GW // PT):
                        ps = p1p.tile([PT, TP], f32, tag="proj")
                        for (c0, cn) in _chunks(0):
                            for k in range(KT):
                                nc.tensor.matmul(
                                    ps[:, c0:c0 + cn],
                                    wts[wname][k][:, m * PT:(m + 1) * PT],
                                    xt[k][:, c0:c0 + cn],
                                    start=(k == 0), stop=(k == KT - 1))
                        nc.scalar.add(dest[m][:], ps[:], biases[m][:])

                # v: out (tsz, 512) per t-tile
                for t in range(NT):
                    tsz = _tsz(t)
                    ps = p1vp.tile([PT, GW], f32, tag="vproj")
                    for k in range(KT):
                        nc.tensor.matmul(
                            ps[:tsz, :],
                            xt[k][:, t * PT:t * PT + tsz],
                            wts["wv"][k][:],
                            start=(k == 0), stop=(k == KT - 1))
                    nc.scalar.copy(
                        vt[t][:tsz, :, 0:HD],
                        ps[:tsz, :].rearrange("p (h d) -> p h d", h=HPG))

            # ================= phase 2: attention =================
            with tc.tile_pool(name="ht", bufs=10) as htp, \
                 tc.tile_pool(name="ax", bufs=4) as axp, \
                 tc.tile_pool(name="bc", bufs=2) as bcp, \
                 tc.tile_pool(name="rc", bufs=2) as rcp, \
                 tc.tile_pool(name="attp", bufs=4, space="PSUM") as attp, \
                 tc.tile_pool(name="yp", bufs=2, space="PSUM") as yp:

                def _mask_regions(s, c0, cn):
                    # (global_lo, global_hi, mask_tile, mask_col0, row_hi)
                    regs = []
                    if s in (0, 1):
                        regs.append((285, 541, msk[("m01", s)], 285, PT))
                        regs.append((571, T, msk[("m02", s)], 571, PT))
                    elif s in (2, 3):
                        regs.append((571, T, msk[("m12", s - 2)], 571, PT))
                    elif s == 4:
                        regs.append((571, T, msk[("m12", 2)], 571, 32))
                    out = []
                    for (lo, hi, mtile, m0, rhi) in regs:
                        a, b = max(lo, c0), min(hi, c0 + cn)
                        if a < b:
                            out.append((a, b, mtile, m0, rhi))
                    return out

                def att_stage(h, s, y_ps):
                    """One (head, s-tile), pipelined per single-bank chunk."""
                    mt, p0 = h // 2, (h % 2) * HD
                    ssz = _tsz(s)
                    t0 = s * PT
                    base = _base(t0)
                    ht_t = htp.tile([PT, TP], hdt, tag="ht")
                    nc.sync.dma_start(out=ht_t[:ssz, 0:TP - base],
                                      in_=hT[h, t0:t0 + ssz, base:TP])
                    a_sb = axp.tile([PT, TP], mdt, tag="ax")
                    for (c0, cn) in _chunks_w(t0):
                        a_ps = attp.tile([PT, BANK], f32, tag="att")
                        qc0 = max(c0, t0) if QNARROW else c0
                        nc.tensor.matmul(
                            a_ps[:ssz, qc0 - c0:cn],
                            kTt[mt][p0:p0 + HD, t0:t0 + ssz],
                            qT[mt][p0:p0 + HD, qc0:c0 + cn],
                            start=True, stop=(H_ON_DVE))
                        if H_ON_DVE:
                            nc.vector.tensor_add(
                                a_ps[:ssz, 0:cn], a_ps[:ssz, 0:cn],
                                ht_t[:ssz, c0 - base:c0 - base + cn])
                        else:
                            nc.tensor.matmul(
                                a_ps[:ssz, 0:cn],
                                id_sb[:ssz, :ssz],
                                ht_t[:ssz, c0 - base:c0 - base + cn],
                                start=False, stop=True)
                        for (a, b, mtile, m0, rhi) in _mask_regions(s, c0, cn):
                            nc.vector.tensor_mul(
                                a_ps[0:rhi, a - c0:b - c0],
                                a_ps[0:rhi, a - c0:b - c0],
                                mtile[0:rhi, a - m0:b - m0])
                        nc.scalar.activation(a_sb[:ssz, c0 - base:c0 - base + cn],
                                             a_ps[:ssz, 0:cn], Exp)
                        # bank 0 of y_ps last gets fed at s=3 (t0=384<512)
                        last_s = (BANK // PT - 1) if c0 < BANK else (NT - 1)
                        yc0 = max(c0, t0) if YNARROW else c0
                        ycn = cn - (yc0 - c0)
                        nc.tensor.matmul(
                            y_ps[:, yc0:yc0 + ycn],
                            vt[s][:ssz, h % HPG, :],
                            a_sb[:ssz, yc0 - base:yc0 - base + ycn],
                            start=(s == 0), stop=(s == last_s))

                def head_tail(h, y_ps):
                    mt, p0 = h // 2, (h % 2) * HD
                    recip = rcp.tile([1, TP], mdt, tag="rc")
                    with nc.allow_low_precision(reason="fp32r recip feeds "
                                                "full-rate fp32r bcast mm"):
                        nc.vector.reciprocal(recip[:], y_ps[HD:HD + 1, :])
                    b_sb = bcp.tile([HD, TP], f32, tag="bc")
                    for (c0, cn) in _chunks(0):
                        b_ps = attp.tile([HD, BANK], f32, tag="att")
                        nc.tensor.matmul(b_ps[:, 0:cn], ones64[:],
                                         recip[:, c0:c0 + cn],
                                         start=True, stop=True)
                        nc.scalar.copy(b_sb[:, c0:c0 + cn], b_ps[:, 0:cn])
                    nc.vector.tensor_mul(yT[mt][p0:p0 + HD, :], y_ps[0:HD, :],
                                         b_sb[:])

                for hp in range(HPG // 2):
                    hA, hB = 2 * hp, 2 * hp + 1
                    yA = yp.tile([VW, TP], f32, tag="y")
                    yB = yp.tile([VW, TP], f32, tag="y")
                    for s in range(NT):
                        att_stage(hA, s, yA)
                        att_stage(hB, s, yB)
                    head_tail(hA, yA)
                    head_tail(hB, yB)

            # ================= phase 3: output projection =================
            with tc.tile_pool(name="p3o", bufs=2) as p3o, \
                 tc.tile_pool(name="p3p", bufs=3, space="PSUM") as p3p:
                for m in range(C // PT):
                    ps = p3p.tile([PT, TP], f32, tag="op")
                    for (c0, cn) in _chunks(0):
                        for k in range(GW // PT):
                            nc.tensor.matmul(
                                ps[:, c0:c0 + cn],
                                wpt[k][:, m * PT:(m + 1) * PT],
                                yT[k][:, c0:c0 + cn],
                                start=(k == 0), stop=(k == GW // PT - 1))
                    ot = p3o.tile([PT, TP], f32, tag="ot")
                    nc.scalar.copy(ot[:], ps[:])
                    nc.sync.dma_start(out=outT[m * PT:(m + 1) * PT, :],
                                      in_=ot[:, 0:T])

    with tile.TileContext(nc) as tc:
        if loop_k > 1:
            with tc.For_i(0, loop_k, 1):
                _emit(tc)
        else:
            _emit(tc)

    nc.compile()
    return nc


# ---------------- host-side preprocessing ----------------

def _gauss_A():
    hx = np.arange(7, dtype=np.float32) - 3.0
    k1 = np.exp(-0.5 * (hx / 1.5) ** 2)
    k1 = (k1 / k1.sum()).astype(np.float32)
    A = np.zeros((16, 16), np.float32)
    for i in range(16):
        for u in range(7):
            p = i - 3 + u
            if p < 0:
                p = -p
            if p > 15:
                p = 30 - p
            A[i, p] += k1[u]
    return A


def _blurred_map(f, b_perm):
    # f, b_perm: (B, 256, 256) -> reference's _blurred_map in numpy
    A = _gauss_A()
    bi = (f * b_perm).reshape(B * 256, 16, 16)
    bl = np.einsum("ij,njk,lk->nil", A, bi, A, optimize=True).astype(np.float32)
    mn, mx = bl.min(), bl.max()
    bl = np.clip((bl - mn) / (mx - mn), 0.0, 1.0)
    return bl.reshape(B, 256, 256) * f * b_perm


def _h_cast(a):
    if H_BF16:
        import ml_dtypes
        return np.ascontiguousarray(a).astype(ml_dtypes.bfloat16)
    return np.ascontiguousarray(a)


def _prep_inputs(x, h, f01, f02, f12, b01, b02, b12,
                 Wq, bq, Wk, bk, Wv, bv, Wp, bp):
    blur01 = _blurred_map(f01, np.transpose(b01, (0, 2, 1)))
    blur02 = _blurred_map(f02, np.transpose(b02, (0, 2, 1)))
    blur12 = _blurred_map(f12, np.transpose(b12, (0, 2, 1)))

    # h^T padded to TP rows/cols; the whole sub-diagonal (t < s) plus the
    # padding row/column are -1e30 so exp kills everything non-causal,
    # including sub-diagonal columns the widened fp32r chunks compute.
    hTfull = np.full((B, NH, TP, TP), NEG, np.float32)
    hTfull[:, :, :T, :T] = np.transpose(h, (0, 1, 3, 2))
    tri = np.tril(np.ones((TP, TP), dtype=bool), -1)  # t < s
    hTfull[:, :, tri] = NEG
    # padding column t=827 stays finite (exp=1) so its softmax sum is nonzero
    # and the never-stored column produces no inf/NaN downstream
    hTfull[:, :, :, T] = 0.0

    cstv = np.zeros((PT, HD + 1), np.float32)
    cstv[:, 0:HD] = 1.0
    identv = np.eye(PT, dtype=np.float32)

    in_maps = []
    for c in range(NCORES):
        b, g = c // 2, c % 2
        sl = slice(g * GW, (g + 1) * GW)
        m12p = np.ones((384, 256), np.float32)
        m12p[30:286, :] = blur12[b].T
        in_maps.append({
            "xT": np.ascontiguousarray(x[b].T),
            "wq": np.ascontiguousarray(Wq[:, sl]) / 8.0,
            "wk": np.ascontiguousarray(Wk[:, sl]),
            "wv": np.ascontiguousarray(Wv[:, sl]),
            "wp": np.ascontiguousarray(Wp[sl, :]),
            "bq": (bq[sl] / 8.0).reshape(GW, 1).astype(np.float32),
            "bk": bk[sl].reshape(GW, 1).astype(np.float32),
            "hT": _h_cast(hTfull[b, g * HPG:(g + 1) * HPG]),
            "m01": np.ascontiguousarray(blur01[b].T.reshape(2, PT, 256)),
            "m02": np.ascontiguousarray(blur02[b].T.reshape(2, PT, 256)),
            "m12": np.ascontiguousarray(m12p.reshape(3, PT, 256)),
            "cst": cstv,
            "ident": identv,
        })
    return in_maps


def _postprocess(results, Wv_bias_row):
    out = np.empty((B, T, C), np.float32)
    for b in range(B):
        acc = results[2 * b]["outT"] + results[2 * b + 1]["outT"]
        out[b] = acc.T + Wv_bias_row
    return out


def kernel(**inputs):
    inputs = {k: np.asarray(v, dtype=np.float32) for k, v in inputs.items()}
    if "nc" not in _CACHE:
        _CACHE["nc"] = _build_nc()
    nc = _CACHE["nc"]

    in_maps = _prep_inputs(**inputs)
    from concourse import bass_utils
    res = bass_utils.run_bass_kernel_spmd(nc, in_maps,
                                          core_ids=list(range(NCORES)))
    row = inputs["bv"] @ inputs["Wp"] + inputs["bp"]
    return _postprocess(res.results, row.astype(np.float32))
